# revision 16
# baseline (speedup 1.0000x reference)
"""Trainium2 Bass kernel for the two-level softmax-pooled text/video retrieval head.

Computes, for text_feat [256,32,512], video_feat [256,16,512], text_mask [256,32]:
    out[a,b] = (t2v(a,b) + v2t(a,b)) / 2
where t2v/v2t are two-level softmax-weighted poolings of the cross token/frame
cosine similarity tensor logits[a,b,t,v] (see reference module).

Sharding: text axis A split across 8 NeuronCores (32 queries each); video
features replicated. Host does l2-normalization + transposition (layout prep);
the device does all einsum + softmax compute.

v7 design (per core: A_loc=32, T=32 -> 1024 (q,t) rows; B=256, V=16 -> 4096
(b,v) cols; D=512). The three engines are near-evenly loaded (~74-84 us busy
each), so the kernel keeps all of PE/ACT/DVE saturated:
  - inputs are bf16 (0.4% feature rounding, ~1.7% first-level softmax weight
    noise -- well inside the 2e-2 tolerance); halves input DMA.
  - m-tiles are processed in PAIRS: one [128,1024] PSUM tile (2 banks) per
    pair, so ACT exp / ACT X-copy / DVE XE-mult each run at [128,1024] width
    (halves per-instruction overhead).
  - E and XE are bf16; the XE multiply runs in the DVE 2x_1p mode.
  - sum-over-v (t2v level 1) is a 4-step strided add-tree on DVE (bf16 2x);
    a single tensor_reduce has no fast mode and is ~1.7x slower.
  - sum-over-t (v2t level 1 den/num) stays on PE as mask-valued selector
    matmuls (bf16, exact 0/1, 32-wide stationary -> region-scoped PSUM
    accumulation), software-pipelined two tiles behind the main matmuls so
    PE never waits on ACT/DVE.
  - loop order is pair-outer / n-tile-inner within each b-half, so the t2v
    second level for a pair runs as soon as its half's columns are done --
    nearly all of phase 2 overlaps the main loop, shrinking the serial tail.
"""

import os
import sys

import numpy as np

if "/opt/trn_rl_repo" not in sys.path:
    sys.path.insert(0, "/opt/trn_rl_repo")

A, T_TOK, B, V_FRM, D = 256, 32, 256, 16, 512
N_CORES = 8
A_LOC = A // N_CORES            # 32 queries per core
M_ROWS = A_LOC * T_TOK          # 1024  (q,t) rows
N_COLS = B * V_FRM              # 4096  (b,v) cols
N_MT = M_ROWS // 128            # 8 M-tiles (4 queries each)
N_PAIR = N_MT // 2              # 4 M-pairs
N_NT = N_COLS // 512            # 8 N-tiles (32 videos each)
N_KC = D // 128                 # 4 K-chunks
TAU = 100.0
SHIFT = -30.0                   # global exp shift (softmax-invariant)
EPS = 1e-6

_PROGRAM_CACHE = {}


def _build_program(reps=1, variant=0):
    import contextlib

    import concourse.mybir as mybir
    import concourse.tile as tile
    from concourse import bacc

    f32 = mybir.dt.float32
    bf16 = mybir.dt.bfloat16
    EXP = mybir.ActivationFunctionType.Exp
    CPY = mybir.ActivationFunctionType.Copy
    MUL = mybir.AluOpType.mult
    ADD = mybir.AluOpType.add

    nc = bacc.Bacc("TRN2", target_bir_lowering=False, debug=False)

    tT_d = nc.dram_tensor("tT", [D, M_ROWS], bf16, kind="ExternalInput")
    vT_d = nc.dram_tensor("vT", [D, N_COLS], bf16, kind="ExternalInput")
    sel_d = nc.dram_tensor("sel", [128, N_MT * 32], bf16, kind="ExternalInput")
    # bias width varies with reps/variant so each build gets a distinct HLO
    # hash (the NEFF cache otherwise silently reuses the first-compiled
    # program)
    bias_cols = 1 + (reps - 1) + 7 * variant
    bias_d = nc.dram_tensor("bias", [128, bias_cols], f32, kind="ExternalInput")
    out_d = nc.dram_tensor("out", [A_LOC, B], f32, kind="ExternalOutput")

    with tile.TileContext(nc) as tc, contextlib.ExitStack() as ctx:
        persist = ctx.enter_context(tc.tile_pool(name="persist", bufs=1))
        ps_pool = ctx.enter_context(tc.tile_pool(name="ps", bufs=2, space="PSUM"))
        dn2_pool = ctx.enter_context(tc.tile_pool(name="dn2", bufs=1, space="PSUM"))
        dn3_pool = ctx.enter_context(tc.tile_pool(name="dn3", bufs=1, space="PSUM"))
        e_pool = ctx.enter_context(tc.tile_pool(name="e", bufs=6))
        tr_pool = ctx.enter_context(tc.tile_pool(name="tr", bufs=3))
        xb_pool = ctx.enter_context(tc.tile_pool(name="xb", bufs=3))
        t2v_pool = ctx.enter_context(tc.tile_pool(name="t2v", bufs=2))
        w_pool = ctx.enter_context(tc.tile_pool(name="w", bufs=3))
        v_pool = ctx.enter_context(tc.tile_pool(name="v2", bufs=2))

        # ---- persistent inputs. Emission order == DMA priority: the first
        # pair's K-chunks interleaved with the n=0 video tile so PE can start
        # ~2us in, then the small constants, then the rest in consumption
        # order.
        tt_tiles = []
        vt_tiles = {}
        for k in range(N_KC):
            t_ = persist.tile([128, M_ROWS], bf16, tag=f"tt_{k}")
            tt_tiles.append(t_)

        # input DMAs alternate between the SP (HWDGE) and the otherwise-idle
        # Pool (SWDGE) issue queues so the startup feed is not serialized on
        # one descriptor generator
        _dma_q = [nc.sync, nc.gpsimd]

        def in_dma(i, out, in_):
            _dma_q[i % 2].dma_start(out=out, in_=in_)

        def vt_load(n):
            for k in range(N_KC):
                t_ = persist.tile([128, 512], bf16, tag=f"vt_{k}_{n}")
                in_dma(k, t_[:],
                       vT_d.ap()[128 * k:128 * (k + 1), 512 * n:512 * (n + 1)])
                vt_tiles[(k, n)] = t_

        # first pair's text chunks interleaved with the n=0 video tile, then
        # the remaining n-tiles in the order the mp-outer loop consumes them
        for k in range(N_KC):
            in_dma(0, tt_tiles[k][:, 0:512],
                   tT_d.ap()[128 * k:128 * (k + 1), 0:512])
            t_ = persist.tile([128, 512], bf16, tag=f"vt_{k}_0")
            in_dma(1, t_[:], vT_d.ap()[128 * k:128 * (k + 1), 0:512])
            vt_tiles[(k, 0)] = t_
        vt_load(1)
        bias_sb = persist.tile([128, bias_cols], f32, tag="bias")
        nc.sync.dma_start(out=bias_sb[:], in_=bias_d.ap())
        sel_sb = persist.tile([128, N_MT * 32], bf16, tag="sel")
        nc.sync.dma_start(out=sel_sb[:], in_=sel_d.ap())
        vt_load(2)
        vt_load(3)
        for k in range(N_KC):
            in_dma(k, tt_tiles[k][:, 512:1024],
                   tT_d.ap()[128 * k:128 * (k + 1), 512:1024])
        for n in range(4, N_NT):
            vt_load(n)

        # S|N accumulator, side-major: col = side*2048 + m*256 + n*32 + b
        sn_all = persist.tile([128, 2 * N_MT * 256], f32, tag="sn_all")
        # final staging, (j,q)-partition layout: row = 32j+q, col = 32h+b'
        # (video index = 128h + 32j + b'); avoids any partition-crossing DMA
        # until the single strided output DMA
        vt2_full = persist.tile([128, 64], f32, tag="vt2_full")
        bias0 = bias_sb[:, 0:1]

        def issue_sel(den2, num2, item):
            """The 4 over-t selector matmuls for one queued pair-tile.
            32-wide stationary: band j of den2/num2 is its own region-scoped
            accumulation group, complete at (mp==3, mi==1)."""
            j, mp, exe = item
            for mi in range(2):
                m = 2 * mp + mi
                selm = sel_sb[:, 32 * m:32 * (m + 1)]
                band = slice(32 * j, 32 * (j + 1))
                nc.tensor.matmul(
                    den2[band, :], selm, exe[:, 512 * mi:512 * (mi + 1)],
                    start=(mp == 0 and mi == 0), stop=(mp == N_PAIR - 1 and mi == 1),
                    skip_group_check=True, tile_position=(0, 32 * j),
                )
                nc.tensor.matmul(
                    num2[band, :], selm, exe[:, 1024 + 512 * mi:1536 + 512 * mi],
                    start=(mp == 0 and mi == 0), stop=(mp == N_PAIR - 1 and mi == 1),
                    skip_group_check=True, tile_position=(0, 32 * j),
                )

        for _rep in range(reps):
            # t2v second-level accumulators in the (j,q)/(h,b') layout:
            # row = 32j+q, col = 32h+b'
            den3 = dn3_pool.tile([128, 64], f32, tag="den3")
            num3 = dn3_pool.tile([128, 64], f32, tag="num3")

            # --- phase-2 (t2v second level) for one m-tile and one b-half;
            # interleaved into the main loop right where its inputs are
            # ready.  Split into an a-stage (DVE ratio + ACT exp) and a
            # b-stage (DVE weight product + PE matmuls) so tail emission can
            # interleave several chains without idling DVE on ACT hops.
            def t2v_level2_a(m, h):
                mb, hb = m * 256, 128 * h
                s1 = sn_all[:, mb + hb:mb + hb + 128]
                n1 = sn_all[:, 2048 + mb + hb:2048 + mb + hb + 128]
                rs1 = t2v_pool.tile([128, 128], f32, tag="rs1")
                nc.vector.reciprocal(rs1[:], s1)
                t2v_t = t2v_pool.tile([128, 128], f32, tag="t2v_t")
                nc.vector.tensor_tensor(t2v_t[:], n1, rs1[:], op=MUL)
                w_t = w_pool.tile([128, 128], bf16, tag="w_t")
                nc.scalar.activation(w_t[:], t2v_t[:], EXP, bias=bias0,
                                     scale=TAU)
                return t2v_t, w_t

            def t2v_level2_b(m, h, t2v_t, w_t):
                xw_t = w_pool.tile([128, 128], bf16, tag="xw_t")
                with nc.allow_low_precision(reason="bf16 softmax weights"):
                    nc.vector.tensor_tensor(xw_t[:], t2v_t[:], w_t[:], op=MUL)
                selm = sel_sb[:, 32 * m:32 * (m + 1)]
                for j in range(4):
                    band = slice(32 * j, 32 * (j + 1))
                    hcol = slice(32 * h, 32 * h + 32)
                    wcol = slice(32 * j, 32 * (j + 1))
                    nc.tensor.matmul(
                        den3[band, hcol], selm, w_t[:, wcol],
                        start=(m == 0), stop=(m == N_MT - 1),
                        skip_group_check=True, tile_position=(0, 32 * j),
                    )
                    nc.tensor.matmul(
                        num3[band, hcol], selm, xw_t[:, wcol],
                        start=(m == 0), stop=(m == N_MT - 1),
                        skip_group_check=True, tile_position=(0, 32 * j),
                    )

            def t2v_level2(m, h):
                t2v_t, w_t = t2v_level2_a(m, h)
                t2v_level2_b(m, h, t2v_t, w_t)

            # ---- main loop: halves (b 0:128 / 128:256) x M-pairs x N-tiles
            for h in range(2):
                den2 = dn2_pool.tile([128, 512], f32, tag="den2")
                num2 = dn2_pool.tile([128, 512], f32, tag="num2")
                sel_queue = []   # software pipeline: sel MMs run 2 tiles late
                for mp in range(N_PAIR):
                    for j in range(4):
                        n = 4 * h + j
                        ps = ps_pool.tile([128, 1024], f32, tag="ps")
                        for mi in range(2):
                            m = 2 * mp + mi
                            for k in range(N_KC):
                                nc.tensor.matmul(
                                    ps[:, 512 * mi:512 * (mi + 1)],
                                    tt_tiles[k][:, 128 * m:128 * (m + 1)],
                                    vt_tiles[(k, n)][:],
                                    start=(k == 0),
                                    stop=(k == N_KC - 1),
                                )
                        # E|XE for the pair: [E(m0) E(m1) XE(m0) XE(m1)]
                        exe = e_pool.tile([128, 2048], bf16, tag="exe")
                        nc.scalar.activation(
                            exe[:, 0:1024], ps[:], EXP, bias=bias0, scale=TAU)
                        # the last pair of the half gates the v2t second
                        # level: multiply straight from PSUM (1x mode) to cut
                        # the ACT X-copy out of the tail's critical chain
                        tail_pair = (mp == N_PAIR - 1 and j == 3)
                        with nc.allow_low_precision(reason="bf16 E/XE"):
                            if tail_pair:
                                nc.vector.tensor_tensor(
                                    exe[:, 1024:2048], ps[:], exe[:, 0:1024],
                                    op=MUL)
                            else:
                                xbt = xb_pool.tile([128, 1024], bf16,
                                                   tag="xbt")
                                nc.scalar.activation(xbt[:], ps[:], CPY)
                                nc.vector.tensor_tensor(
                                    exe[:, 1024:2048], xbt[:], exe[:, 0:1024],
                                    op=MUL)
                        # queue the 4 selector matmuls (over-t sums on PE),
                        # issued 2 tiles later so their ACT/DVE deps are done
                        sel_queue.append((j, mp, exe))
                        if len(sel_queue) > 2:
                            issue_sel(den2, num2, sel_queue.pop(0))
                        # sum over v: 4-step strided bf16 add-tree on DVE.
                        # tr: [0:1024]=(c4,b32,v8)  [1024:1536]=(c,b,4)
                        #     [1536:1792]=(c,b,2)
                        tr = tr_pool.tile([128, 1792], bf16, tag="tr")
                        ev16 = exe[:].rearrange("p (c b v) -> p c b v", c=4,
                                                v=16)
                        t8 = tr[:, 0:1024].rearrange("p (c b v) -> p c b v",
                                                     c=4, v=8)
                        t4 = tr[:, 1024:1536].rearrange("p (c b v) -> p c b v",
                                                        c=4, v=4)
                        t2 = tr[:, 1536:1792].rearrange("p (c b v) -> p c b v",
                                                        c=4, v=2)
                        with nc.allow_low_precision(reason="bf16 v-tree"):
                            nc.vector.tensor_tensor(
                                t8[:], ev16[:, :, :, 0:8], ev16[:, :, :, 8:16],
                                op=ADD)
                            nc.vector.tensor_tensor(
                                t4[:], t8[:, :, :, 0:4], t8[:, :, :, 4:8],
                                op=ADD)
                            nc.vector.tensor_tensor(
                                t2[:], t4[:, :, :, 0:2], t4[:, :, :, 2:4],
                                op=ADD)
                        # final add writes fp32 S|N: col = s*2048+m*256+n*32+b
                        snv = sn_all[:].rearrange(
                            "p (s m nb) -> p s m nb", s=2, m=N_MT)[
                            :, :, 2 * mp:2 * mp + 2, 32 * n:32 * (n + 1)]
                        nc.vector.tensor_tensor(
                            snv, t2[:, :, :, 0:1], t2[:, :, :, 1:2], op=ADD)
                    # this pair's half-columns are complete: run its t2v
                    # second level overlapped with the remaining pairs (the
                    # last pair's runs after the v2t chain below, which is
                    # the longer critical path at the end of the half)
                    if mp < N_PAIR - 1:
                        t2v_level2(2 * mp, h)
                        t2v_level2(2 * mp + 1, h)
                while sel_queue:
                    issue_sel(den2, num2, sel_queue.pop(0))

                # ---- second level of v2t for this half (softmax over v),
                # split into two column groups (b' 0:16 / 16:32) so the serial
                # chain pipelines and the last-half tail is ~2x shorter ----
                rden2 = v_pool.tile([128, 512], f32, tag="rden2")
                v_t = v_pool.tile([128, 512], f32, tag="v_t")
                exev = v_pool.tile([128, 1024], bf16, tag="exev")
                trv = v_pool.tile([128, 896], bf16, tag="trv")
                snv_t = v_pool.tile([128, 64], f32, tag="snv_t")
                rsv_t = v_pool.tile([128, 32], f32, tag="rsv_t")
                ev16 = exev[:].rearrange("p (c b v) -> p c b v", c=2, v=16)
                t8 = trv[:, 0:512].rearrange("p (c b v) -> p c b v", c=2, v=8)
                t4 = trv[:, 512:768].rearrange("p (c b v) -> p c b v", c=2, v=4)
                t2 = trv[:, 768:896].rearrange("p (c b v) -> p c b v", c=2, v=2)
                sn8 = snv_t[:].rearrange("p (c b) -> p c b", c=2)

                def v2t_a(cg):
                    cs = slice(256 * cg, 256 * cg + 256)
                    nc.vector.reciprocal(rden2[:, cs], den2[:, cs])
                    nc.vector.tensor_tensor(
                        v_t[:, cs], num2[:, cs], rden2[:, cs], op=MUL)
                    nc.scalar.activation(
                        exev[:, cs], v_t[:, cs], EXP, bias=bias0, scale=TAU)

                def v2t_b(cg):
                    cs = slice(256 * cg, 256 * cg + 256)
                    bs = slice(16 * cg, 16 * cg + 16)
                    with nc.allow_low_precision(reason="bf16 Ev/XEv"):
                        nc.vector.tensor_tensor(
                            exev[:, 512 + 256 * cg:768 + 256 * cg],
                            v_t[:, cs], exev[:, cs], op=MUL)
                        nc.vector.tensor_tensor(
                            t8[:, :, bs, :], ev16[:, :, bs, 0:8],
                            ev16[:, :, bs, 8:16], op=ADD)
                        nc.vector.tensor_tensor(
                            t4[:, :, bs, :], t8[:, :, bs, 0:4],
                            t8[:, :, bs, 4:8], op=ADD)
                        nc.vector.tensor_tensor(
                            t2[:, :, bs, :], t4[:, :, bs, 0:2],
                            t4[:, :, bs, 2:4], op=ADD)
                    nc.vector.tensor_tensor(
                        sn8[:, :, bs], t2[:, :, bs, 0:1], t2[:, :, bs, 1:2],
                        op=ADD)
                    nc.vector.reciprocal(rsv_t[:, bs], snv_t[:, bs])
                    # vt2 = 0.5 * Nv / Sv  (the final /2 folded in here),
                    # written straight into the (j,q)/(h,b') staging tile
                    nc.vector.scalar_tensor_tensor(
                        out=vt2_full[:, 32 * h + 16 * cg:32 * h + 16 * cg + 16],
                        in0=snv_t[:, 32:64][:, bs], scalar=0.5,
                        in1=rsv_t[:, bs], op0=MUL, op1=MUL,
                    )

                # interleave the v2t chains, the deferred last pair's t2v
                # level 2, and phase 3 so DVE always has ready work while
                # ACT exps round-trip
                m_a, m_b = 2 * (N_PAIR - 1), 2 * N_PAIR - 1
                v2t_a(0)
                ta = t2v_level2_a(m_a, h)
                tb = t2v_level2_a(m_b, h)
                v2t_a(1)
                v2t_b(0)
                t2v_level2_b(m_a, h, *ta)
                t2v_level2_b(m_b, h, *tb)
                v2t_b(1)

                # ---- phase 3 for this half: t2v2 = 0.5*Num3/Den3, combine
                # with v2t, and DMA out[q, 128h+32j+b'].  den3/num3's h-region
                # groups stop at m==7 just above, so the h=0 finale fully
                # overlaps the h=1 main loop. ----
                hcol = slice(32 * h, 32 * h + 32)
                rden3 = t2v_pool.tile([128, 32], f32, tag="rden3")
                nc.vector.reciprocal(rden3[:], den3[:, hcol])
                t2v2 = t2v_pool.tile([128, 32], f32, tag="t2v2")
                nc.vector.scalar_tensor_tensor(
                    out=t2v2[:], in0=num3[:, hcol], scalar=0.5, in1=rden3[:],
                    op0=MUL, op1=MUL,
                )
                out_sb = t2v_pool.tile([128, 32], f32, tag="out_sb")
                nc.vector.tensor_tensor(out_sb[:], t2v2[:],
                                        vt2_full[:, hcol], op=ADD)
                out_ap = out_d.ap().rearrange(
                    "q (h j b) -> h j q b", h=2, j=4)[h:h + 1]
                nc.sync.dma_start(out=out_ap, in_=out_sb[:])

    nc.compile()
    return nc


def _get_program(reps=1, variant=0, **_ignored):
    key = (reps, variant)
    if key not in _PROGRAM_CACHE:
        _PROGRAM_CACHE[key] = _build_program(reps, variant)
    return _PROGRAM_CACHE[key]


def _l2norm(a):
    n = np.linalg.norm(a, axis=-1, keepdims=True)
    return a / np.maximum(n, EPS)


def _bf16(a):
    import ml_dtypes
    return a.astype(ml_dtypes.bfloat16)


def prepare_inputs(text_feat, video_feat, text_mask):
    """Host-side shard/layout prep. Returns in_maps for the 8 cores."""
    t = _l2norm(text_feat.astype(np.float32))          # [A, T, D]
    v = _l2norm(video_feat.astype(np.float32))         # [B, V, D]
    mask = text_mask.astype(np.float32)

    # video: [B, V, D] -> [D, B*V], shared by all cores
    vT = _bf16(np.ascontiguousarray(v.reshape(B * V_FRM, D).T))

    p = np.arange(128)
    in_maps = []
    for c in range(N_CORES):
        tc_ = t[c * A_LOC:(c + 1) * A_LOC]             # [32, T, D]
        tT = _bf16(np.ascontiguousarray(tc_.reshape(M_ROWS, D).T))  # [D, 1024]
        mk = mask[c * A_LOC:(c + 1) * A_LOC]           # [32, T]
        # the selector carries the 0/1 mask values: padded tokens contribute
        # exactly 0 to every partition-direction (over-t) sum
        sel = np.zeros((128, N_MT * 32), np.float32)
        for m in range(N_MT):
            mvals = mk[4 * m:4 * m + 4].reshape(128)   # mask for rows of tile m
            sel[p, m * 32 + 4 * m + p // 32] = mvals
        bias = np.full((128, 1), SHIFT, np.float32)
        in_maps.append({"tT": tT, "vT": vT, "sel": _bf16(sel), "bias": bias})
    return in_maps


def run(in_maps, trace=False, reps=1, variant=0, **kwargs):
    import concourse.mybir as mybir
    from concourse import bass_utils

    nc = _get_program(reps=reps, variant=variant)
    # pad inputs to the program's declared shapes (bias width varies by build)
    shapes = {}
    for alloc in nc.m.functions[0].allocations:
        if isinstance(alloc, mybir.MemoryLocationSet) and alloc.kind == "ExternalInput":
            shapes[alloc.memorylocations[0].name] = tuple(alloc.tensor_shape)
    fixed = []
    for m in in_maps:
        mm = {}
        for k, v in m.items():
            shp = shapes.get(k, tuple(v.shape))
            if tuple(v.shape) != shp:
                out = np.full(shp, SHIFT if k == "bias" else 0.0, v.dtype)
                sl = tuple(slice(0, min(s, t)) for s, t in zip(v.shape, shp))
                out[sl] = v[sl]
                mm[k] = out
            else:
                mm[k] = v
        fixed.append(mm)
    return bass_utils.run_bass_kernel_spmd(
        nc, fixed, core_ids=list(range(N_CORES)), trace=trace, **kwargs
    )


def kernel(text_feat, video_feat, text_mask):
    in_maps = prepare_inputs(
        np.asarray(text_feat), np.asarray(video_feat), np.asarray(text_mask)
    )
    res = run(in_maps)
    out = np.concatenate([res.results[c]["out"] for c in range(N_CORES)], axis=0)
    return out.astype(np.float32)
